# revision 9
# baseline (speedup 1.0000x reference)
"""Trainium2 Bass kernel for a 2-layer GAT (B=8, N=1024, F=256, D=64, H=8, C=256).

Sharding: data-parallel over batch — one batch element per NeuronCore (8 cores).

Key trick vs naive: the GAT score nonlinearity is separable through the
softmax. With s_ij = sl_i + sr_j and LR = LeakyReLU(0.2):
    exp(LR(s)) = e^{sl_i} * Q_j * max(1, u_i * v_j)
where u = e^{-0.8 sl}, v = e^{-0.8 sr}, Q = e^{sr}   (exp is monotone, so
exp(max(s, .2s)) = max(e^s, e^{.2s}) = e^s * max(1, e^{-.8s}), and e^{-.8s}
factors). The e^{sl_i} factor is constant per softmax row and cancels in
num/den. So NO exp/LeakyReLU over the N^2 score matrix on-chip: per head
    eT[j,i] = max(u_i * v_j, 1) * m[j,i]     (one 2-op tensor_scalar + one
                                              tensor_tensor on DVE)
and Q_j is folded into the aggregation operand (Qh, ones-col -> Q gives the
softmax denominator). u/v/Q for layer 1 are host-precomputed per-node; for
layer 2 they are computed on-chip with 3 tiny ACT exps from tl/tr.

Per-core algorithm:
  h      = x @ W_all (+Wb)                          [n, 512]   PE, bf16
  hx     = [Q_j*h | Q_j | pad] per head             [j, 8*66]  exit+scale
  eT     = max(ub*v_j, 1) * msk    per head         [j, NCH*N] DVE only
  agg    : num/den via eT-stationary matmuls        PE
  hh     = num * (1/den); z = ELU(hh)
  layer 2 identical with g = z @ Wo (+tl/tr cols), u2/v2/Q2 from ACT exps
  out    = ELU(a2 @ g / Z2) + x
"""

import numpy as np
import ml_dtypes
from contextlib import ExitStack

BF16 = ml_dtypes.bfloat16
B, N, F, D, H, C = 8, 1024, 256, 64, 8, 256
HD = H * D  # 512
ALPHA = 0.2

_CACHE = {}


def _build_program():
    import concourse.bacc as bacc
    import concourse.bass as bass
    import concourse.mybir as mybir
    from concourse.tile import TileContext
    from concourse.masks import make_identity

    dt = mybir.dt
    Alu = mybir.AluOpType
    Act = mybir.ActivationFunctionType

    nc = bacc.Bacc()

    xt = nc.declare_dram_parameter("xt", [F + 1, N], dt.bfloat16, isOutput=False)
    xs = nc.declare_dram_parameter("xs", [N, F], dt.float32, isOutput=False)
    msk = nc.declare_dram_parameter("msk", [N, N], dt.bfloat16, isOutput=False)
    wp = nc.declare_dram_parameter("wp", [F + 1, HD], dt.bfloat16, isOutput=False)
    ut = nc.declare_dram_parameter("ut", [H, N], dt.bfloat16, isOutput=False)
    vq = nc.declare_dram_parameter("vq", [N, 2 * H], dt.float32, isOutput=False)
    wo = nc.declare_dram_parameter("wo", [HD + 1, C + 2], dt.bfloat16, isOutput=False)
    out_d = nc.declare_dram_parameter("out", [N, C], dt.float32, isOutput=True)

    rows_d = nc.dram_tensor("rows_bounce", [1, N], dt.bfloat16)

    NCH = N // 128  # 8 chunks of 128 nodes

    def bcast128(row_ap):
        # [1, N] DRAM row -> [128, N] partition-broadcast read for DMA
        return bass.AP(
            tensor=row_ap.tensor,
            offset=row_ap.offset,
            ap=[[0, 128]] + list(row_ap.ap),
        )

    with TileContext(nc) as tc:
        with ExitStack() as ctx:
            cons = ctx.enter_context(tc.tile_pool(name="cons", bufs=1))
            bc = ctx.enter_context(tc.tile_pool(name="bc", bufs=3))
            eb = ctx.enter_context(tc.tile_pool(name="eb", bufs=6))
            wk = ctx.enter_context(tc.tile_pool(name="wk", bufs=3))
            sm = ctx.enter_context(tc.tile_pool(name="sm", bufs=3))
            pmm = ctx.enter_context(tc.tile_pool(name="pmm", bufs=4, space="PSUM"))
            pm2 = ctx.enter_context(tc.tile_pool(name="pm2", bufs=2, space="PSUM"))
            ptp = ctx.enter_context(tc.tile_pool(name="ptp", bufs=1, space="PSUM"))

            # ---------- constants / params ----------
            ident_f = cons.tile([128, 128], dt.float32)
            make_identity(nc, ident_f[:, :])
            ident_b = cons.tile([128, 128], dt.bfloat16)
            make_identity(nc, ident_b[:, :])

            # mask first (the first head's tensor_tensor waits on it), split
            # across two engine queues for parallel descriptor generation
            msk_sb = cons.tile([128, NCH * N], dt.bfloat16)
            for c in range(NCH):
                eng = nc.gpsimd if c % 2 == 0 else nc.scalar
                eng.dma_start(
                    out=msk_sb[:, c * N : (c + 1) * N],
                    in_=msk[c * 128 : (c + 1) * 128, :],
                )

            xt_sb = cons.tile([128, 2 * N], dt.bfloat16)
            nc.gpsimd.dma_start(out=xt_sb[:, 0:N], in_=xt[0:128, :])
            nc.gpsimd.dma_start(out=xt_sb[:, N : 2 * N], in_=xt[128:256, :])
            xt_one = cons.tile([1, N], dt.bfloat16)
            nc.gpsimd.dma_start(out=xt_one[:, :], in_=xt[256:257, :])

            wp_sb = cons.tile([128, 2 * HD], dt.bfloat16)
            nc.gpsimd.dma_start(out=wp_sb[:, 0:HD], in_=wp[0:128, :])
            nc.gpsimd.dma_start(out=wp_sb[:, HD : 2 * HD], in_=wp[128:256, :])
            wp_one = cons.tile([1, HD], dt.bfloat16)
            nc.gpsimd.dma_start(out=wp_one[:, :], in_=wp[256:257, :])

            wo_sb = cons.tile([128, 4 * (C + 2)], dt.bfloat16)
            for k in range(4):
                nc.gpsimd.dma_start(
                    out=wo_sb[:, k * (C + 2) : (k + 1) * (C + 2)],
                    in_=wo[k * 128 : (k + 1) * 128, :],
                )
            wo_one = cons.tile([1, C + 2], dt.bfloat16)
            nc.gpsimd.dma_start(out=wo_one[:, :], in_=wo[HD : HD + 1, :])

            # v / Q per-partition scalars: vqt[p, n, 0:8]=v_h, [.., 8:16]=Q_h
            vqt = cons.tile([128, NCH * 2 * H], dt.float32)
            nc.sync.dma_start(
                out=vqt[:, :].rearrange("p (n h) -> p n h", h=2 * H),
                in_=vq[:, :].rearrange("(n p) h -> p n h", p=128),
            )

            # ---------- phase 1: h = x@W_all, scaled by Q ----------
            hx = cons.tile([128, NCH * 8 * 66], dt.bfloat16)  # [Qh(64)|Q|pad]/head
            nc.vector.memset(
                hx[:, :].rearrange("p (n h s) -> p n h s", h=8, s=66)[:, :, :, 64:66],
                1.0,
            )
            qrep = cons.tile([128, NCH * 528], dt.bfloat16)
            for n in range(NCH):
                nc.scalar.activation(
                    qrep[:, n * 528 : (n + 1) * 528].rearrange(
                        "p (h s) -> p h s", s=66
                    ),
                    vqt[:, :]
                    .rearrange("p (n h) -> p n h", h=2 * H)[:, n : n + 1, H : 2 * H]
                    .rearrange("p n h -> p h n")
                    .to_broadcast([128, 8, 66]),
                    Act.Copy,
                )

            def emit_h_proj(n0, n1):
                for n in range(n0, n1):
                    ph = pmm.tile([128, HD], dt.float32, tag="mm")
                    for k in range(2):
                        lt = xt_sb[:, k * N + n * 128 : k * N + n * 128 + 128]
                        nc.tensor.matmul(
                            ph[:, :], lt, wp_sb[:, k * HD : (k + 1) * HD],
                            start=(k == 0), stop=False,
                        )
                    lt1 = xt_one[:, n * 128 : n * 128 + 128]
                    nc.tensor.matmul(
                        ph[:, :], lt1, wp_one[:, :], start=False, stop=True
                    )
                    # exit h -> hx (bf16, 66-stride blocks; Q cols pre-set 1)
                    hx_v = hx[:, n * 528 : (n + 1) * 528].rearrange(
                        "p (h s) -> p h s", s=66
                    )[:, :, 0:64]
                    ph_v = ph[:, :].rearrange("p (h s) -> p h s", s=64)
                    nc.scalar.activation(hx_v, ph_v, Act.Copy)
                    # scale whole 528-block by Q (ones col -> Q, pad -> junk)
                    nc.vector.tensor_tensor(
                        out=hx[:, n * 528 : (n + 1) * 528],
                        in0=hx[:, n * 528 : (n + 1) * 528],
                        in1=qrep[:, n * 528 : (n + 1) * 528],
                        op=Alu.mult,
                    )

            # ---------- phase 2: eT = max(ub*v,1)*msk per head; agg ----------
            zacc = cons.tile([128, NCH * 512], dt.bfloat16)
            rz_all = cons.tile([128, NCH * 8], dt.float32)
            z_sb = cons.tile([128, NCH * HD], dt.bfloat16)

            def build_e(ub, v_col, mask_eng=None):
                """eT[j, i] = max(ub[.,i] * v_j, 1) * msk[j, i]."""
                e = eb.tile([128, NCH * N], dt.bfloat16, tag="e")
                for c in range(NCH):
                    nc.vector.tensor_scalar(
                        out=e[:, c * N : (c + 1) * N], in0=ub[:, :],
                        scalar1=v_col(c), scalar2=1.0,
                        op0=Alu.mult, op1=Alu.max,
                    )
                HN = NCH * N // 2
                me = mask_eng or nc.vector
                for half in range(2):
                    me.tensor_tensor(
                        out=e[:, half * HN : (half + 1) * HN],
                        in0=e[:, half * HN : (half + 1) * HN],
                        in1=msk_sb[:, half * HN : (half + 1) * HN],
                        op=Alu.mult,
                    )
                return e

            for q in range(2):
                ebufs = []
                for hq in range(4):
                    h = 4 * q + hq
                    ub = bc.tile([128, N], dt.bfloat16, tag="ub")
                    nc.sync.dma_start(out=ub[:, :], in_=bcast128(ut[h : h + 1, :]))
                    e = build_e(
                        ub,
                        lambda c, h=h: vqt[:, c * 2 * H + h : c * 2 * H + h + 1],
                        mask_eng=nc.gpsimd if hq >= 2 else None,
                    )
                    ebufs.append(e)
                    if q == 0:
                        emit_h_proj(2 * hq, 2 * hq + 2)

                for ic in range(NCH):
                    pa = pmm.tile([128, 260], dt.float32, tag="mm")
                    for hq in range(4):
                        h = 4 * q + hq
                        e = ebufs[hq]
                        for jc in range(NCH):
                            nc.tensor.matmul(
                                pa[:, hq * 65 : hq * 65 + 65],
                                e[:, jc * N + ic * 128 : jc * N + ic * 128 + 128],
                                hx[:, jc * 528 + h * 66 : jc * 528 + h * 66 + 65],
                                start=(jc == 0),
                                stop=(jc == NCH - 1),
                            )
                    nc.scalar.activation(
                        zacc[:, ic * 512 + q * 256 : ic * 512 + q * 256 + 256]
                        .rearrange("p (h s) -> p h s", s=64),
                        pa[:, :].rearrange("p (h s) -> p h s", s=65)[:, :, 0:64],
                        Act.Copy,
                    )
                    nc.vector.reciprocal(
                        rz_all[:, ic * 8 + q * 4 : ic * 8 + q * 4 + 4]
                        .rearrange("p (h s) -> p h s", s=1),
                        pa[:, :].rearrange("p (h s) -> p h s", s=65)[:, :, 64:65],
                    )

            # ---------- phase 2b-4: per-chunk divide/ELU -> zT -> g ----------
            zt_sb = cons.tile([128, 4 * N], dt.bfloat16)
            zt_one = cons.tile([1, N], dt.bfloat16)
            nc.vector.memset(zt_one[:, :], 1.0)
            gx = cons.tile([128, NCH * 260], dt.bfloat16)
            nc.vector.memset(
                gx[:, :].rearrange("p (n s) -> p n s", s=260)[:, :, 256:257], 1.0
            )
            glgr = cons.tile([128, NCH * 2], dt.float32)
            v2q2 = cons.tile([128, NCH * 2], dt.float32)
            pt2 = ptp.tile([2, N], dt.float32, tag="tp")
            for ic in range(NCH):
                rzrep = wk.tile([128, HD], dt.bfloat16, tag="rzrep")
                hh = wk.tile([128, HD], dt.bfloat16, tag="hh")
                nc.scalar.activation(
                    rzrep[:, :].rearrange("p (h s) -> p h s", s=64),
                    rz_all[:, ic * 8 : (ic + 1) * 8]
                    .rearrange("p (h s) -> p h s", s=1)
                    .to_broadcast([128, 8, 64]),
                    Act.Copy,
                )
                nc.vector.tensor_tensor(
                    out=hh[:, :],
                    in0=zacc[:, ic * 512 : (ic + 1) * 512],
                    in1=rzrep[:, :],
                    op=Alu.mult,
                )
                ee = wk.tile([128, HD], dt.bfloat16, tag="ee")
                nc.scalar.activation(ee[:, :], hh[:, :], Act.Exp)
                r1 = wk.tile([128, HD], dt.bfloat16, tag="r1")
                nc.vector.tensor_scalar(
                    out=r1[:, :], in0=ee[:, :], scalar1=-1.0, scalar2=0.0,
                    op0=Alu.add, op1=Alu.min,
                )
                nc.vector.scalar_tensor_tensor(
                    out=z_sb[:, ic * HD : (ic + 1) * HD],
                    in0=hh[:, :], scalar=0.0, in1=r1[:, :],
                    op0=Alu.max, op1=Alu.add,
                )

                # zT for this node chunk (4 transposed 128x128 blocks)
                pzi = pm2.tile([128, 4 * 128], dt.bfloat16, tag="mm2")
                for kc in range(4):
                    nc.tensor.transpose(
                        pzi[:, kc * 128 : (kc + 1) * 128],
                        z_sb[:, ic * HD + kc * 128 : ic * HD + kc * 128 + 128],
                        ident_b[:, :],
                    )
                nc.scalar.activation(
                    zt_sb[:, :]
                    .rearrange("p (kc n) -> p kc n", n=N)[:, :, ic * 128 : ic * 128 + 128],
                    pzi[:, :].rearrange("p (kc s) -> p kc s", s=128),
                    Act.Copy,
                )

                # g projection for this chunk: g = z@Wo (+tl/tr cols)
                pg = pm2.tile([128, C + 2], dt.float32, tag="mm2")
                for kc in range(4):
                    nc.tensor.matmul(
                        pg[:, :],
                        zt_sb[:, kc * N + ic * 128 : kc * N + ic * 128 + 128],
                        wo_sb[:, kc * (C + 2) : (kc + 1) * (C + 2)],
                        start=(kc == 0), stop=False,
                    )
                nc.tensor.matmul(
                    pg[:, :], zt_one[:, ic * 128 : ic * 128 + 128], wo_one[:, :],
                    start=False, stop=True,
                )
                nc.scalar.activation(
                    gx[:, ic * 260 : ic * 260 + C], pg[:, 0:C], Act.Copy
                )
                nc.vector.tensor_copy(
                    out=glgr[:, ic * 2 : (ic + 1) * 2], in_=pg[:, C : C + 2]
                )
                # v2 = exp(-0.8*tr), Q2 = exp(tr) per-partition scalars
                nc.scalar.activation(
                    v2q2[:, ic * 2 : ic * 2 + 1],
                    glgr[:, ic * 2 + 1 : ic * 2 + 2],
                    Act.Exp, scale=-0.8,
                )
                nc.scalar.activation(
                    v2q2[:, ic * 2 + 1 : ic * 2 + 2],
                    glgr[:, ic * 2 + 1 : ic * 2 + 2],
                    Act.Exp,
                )
                # scale g block by Q2 (ones col -> Q2)
                nc.vector.tensor_scalar(
                    out=gx[:, ic * 260 : ic * 260 + C + 1],
                    in0=gx[:, ic * 260 : ic * 260 + C + 1],
                    scalar1=v2q2[:, ic * 2 + 1 : ic * 2 + 2], scalar2=None,
                    op0=Alu.mult,
                )
                # tl row form for this chunk
                nc.tensor.transpose(
                    pt2[:, ic * 128 : (ic + 1) * 128],
                    glgr[:, ic * 2 : (ic + 1) * 2],
                    ident_f[:, :],
                )

            # u2 row = exp(-0.8 * tl) -> DRAM bounce -> bcast
            u2row = cons.tile([1, N], dt.bfloat16)
            nc.scalar.activation(u2row[:, :], pt2[0:1, :], Act.Exp, scale=-0.8)
            nc.sync.dma_start(out=rows_d[0:1, :], in_=u2row[:, :])

            # ---------- phase 5: output attention layer ----------
            ub2 = bc.tile([128, N], dt.bfloat16, tag="ub")
            nc.sync.dma_start(out=ub2[:, :], in_=bcast128(rows_d[0:1, :]))
            e2 = build_e(ub2, lambda c: v2q2[:, c * 2 : c * 2 + 1])

            for ic in range(NCH):
                po = pmm.tile([128, C + 1], dt.float32, tag="mm")
                for jc in range(NCH):
                    nc.tensor.matmul(
                        po[:, :],
                        e2[:, jc * N + ic * 128 : jc * N + ic * 128 + 128],
                        gx[:, jc * 260 : jc * 260 + C + 1],
                        start=(jc == 0), stop=(jc == NCH - 1),
                    )
                rz2 = sm.tile([128, 1], dt.float32, tag="rz2")
                nc.vector.reciprocal(rz2[:, :], po[:, C : C + 1])
                y = sm.tile([128, C], dt.bfloat16, tag="y")
                nc.vector.tensor_scalar(
                    out=y[:, :], in0=po[:, 0:C], scalar1=rz2[:, :], scalar2=None,
                    op0=Alu.mult,
                )
                e3 = sm.tile([128, C], dt.bfloat16, tag="e3")
                nc.scalar.activation(e3[:, :], y[:, :], Act.Exp)
                r2 = sm.tile([128, C], dt.bfloat16, tag="r2")
                nc.vector.tensor_scalar(
                    out=r2[:, :], in0=e3[:, :], scalar1=-1.0, scalar2=0.0,
                    op0=Alu.add, op1=Alu.min,
                )
                el = sm.tile([128, C], dt.bfloat16, tag="el")
                nc.vector.scalar_tensor_tensor(
                    out=el[:, :], in0=y[:, :], scalar=0.0, in1=r2[:, :],
                    op0=Alu.max, op1=Alu.add,
                )
                xs5 = sm.tile([128, F], dt.float32, tag="xs5")
                nc.sync.dma_start(
                    out=xs5[:, :], in_=xs[ic * 128 : (ic + 1) * 128, :]
                )
                ofin = sm.tile([128, C], dt.float32, tag="ofin")
                nc.vector.tensor_tensor(
                    out=ofin[:, :], in0=el[:, :], in1=xs5[:, :], op=Alu.add,
                )
                nc.sync.dma_start(
                    out=out_d[ic * 128 : (ic + 1) * 128, :], in_=ofin[:, :]
                )

    nc.compile()
    return nc


def get_program():
    if "nc" not in _CACHE:
        _CACHE["nc"] = _build_program()
    return _CACHE["nc"]


def make_in_maps(x, adj, W, Wb, a, ab, Wo, Wob, ao, aob):
    x = np.asarray(x, np.float32)
    adj = np.asarray(adj)
    W = np.asarray(W, np.float32)
    Wb = np.asarray(Wb, np.float32)
    a = np.asarray(a, np.float32)
    ab = np.asarray(ab, np.float32)
    Wo = np.asarray(Wo, np.float32)
    Wob = np.asarray(Wob, np.float32)
    ao = np.asarray(ao, np.float32)
    aob = np.asarray(aob, np.float32)

    # W_all[f, h*D+d] = W[h, f, d];  Wb row flattened the same way
    W_all = W.transpose(1, 0, 2).reshape(F, HD)
    wb_row = Wb.reshape(1, HD)
    wp = np.concatenate([W_all, wb_row], axis=0).astype(BF16)  # [257, 512]

    # sl/sr are tiny per-node linear maps of x — folded on the host.
    # sl[b, h, i] = x[b,i] @ V_l[:,h] + const_l[h]
    # sr[b, h, j] likewise; ab folded into sl
    V_l = np.einsum("hfd,hd->fh", W, a[:, :D]).astype(np.float32)
    V_r = np.einsum("hfd,hd->fh", W, a[:, D:]).astype(np.float32)
    const_l = (Wb * a[:, :D]).sum(1) + ab  # [H]
    const_r = (Wb * a[:, D:]).sum(1)
    sl_all = np.einsum("bnf,fh->bhn", x, V_l) + const_l[None, :, None]  # [B,H,N]
    sr_all = np.einsum("bnf,fh->bnh", x, V_r) + const_r[None, None, :]  # [B,N,H]

    # separable softmax factors (e^{sl} cancels in num/den)
    u_all = np.exp(-0.8 * sl_all)            # [B,H,N]
    v_all = np.exp(-0.8 * sr_all)            # [B,N,H]
    q_all = np.exp(sr_all)                   # [B,N,H]
    vq_all = np.concatenate([v_all, q_all], axis=2).astype(np.float32)  # [B,N,2H]

    u_l = Wo @ ao[:C]  # [512]
    u_r = Wo @ ao[C:]
    wo_top = np.concatenate([Wo, u_l[:, None], u_r[:, None]], axis=1)  # [512, 258]
    wo_bot = np.concatenate(
        [Wob, [Wob @ ao[:C] + aob], [Wob @ ao[C:]]]
    )[None, :]  # [1, 258]
    wo_ext = np.concatenate([wo_top, wo_bot], axis=0).astype(BF16)  # [513, 258]

    ones_row = np.ones((1, N), BF16)
    in_maps = []
    for b in range(B):
        xt = np.concatenate([x[b].T.astype(BF16), ones_row], axis=0)  # [257, 1024]
        mb = np.where(adj[b].T > 0, np.float32(1.0), np.float32(0.0)).astype(BF16)
        in_maps.append(
            {
                "xt": np.ascontiguousarray(xt),
                "xs": np.ascontiguousarray(x[b]),
                "msk": np.ascontiguousarray(mb),
                "wp": wp,
                "ut": np.ascontiguousarray(u_all[b].astype(BF16)),
                "vq": np.ascontiguousarray(vq_all[b]),
                "wo": wo_ext,
            }
        )
    return in_maps


def kernel(**inputs) -> np.ndarray:
    from concourse.bass_utils import run_bass_kernel_spmd

    nc = get_program()
    in_maps = make_in_maps(**inputs)
    res = run_bass_kernel_spmd(nc, in_maps, core_ids=list(range(B)))
    return np.stack([res.results[b]["out"] for b in range(B)], axis=0)


# revision 33
# speedup vs baseline: 1.3429x; 1.3429x over previous
"""Trainium2 Bass kernel for a 2-layer GAT (B=8, N=1024, F=256, D=64, H=8, C=256).

Sharding: data-parallel over batch — one batch element per NeuronCore (8 cores).

Key trick vs naive: the GAT score nonlinearity is separable through the
softmax. With s_ij = sl_i + sr_j and LR = LeakyReLU(0.2):
    exp(LR(s)) = e^{sl_i} * Q_j * max(1, u_i * v_j)
where u = e^{-0.8 sl}, v = e^{-0.8 sr}, Q = e^{sr}   (exp is monotone, so
exp(max(s, .2s)) = max(e^s, e^{.2s}) = e^s * max(1, e^{-.8s}), and e^{-.8s}
factors). The e^{sl_i} factor is constant per softmax row and cancels in
num/den. So NO exp/LeakyReLU over the N^2 score matrix on-chip: per head
    eT[j,i] = max(u_i * v_j, 1) * m[j,i]     (one 2-op tensor_scalar + one
                                              tensor_tensor on DVE)
and Q_j is folded into the aggregation operand (Qh, ones-col -> Q gives the
softmax denominator). u/v/Q for layer 1 are host-precomputed per-node; for
layer 2 they are computed on-chip with 3 tiny ACT exps from tl/tr.

Per-core algorithm:
  h      = x @ W_all (+Wb)                          [n, 512]   PE, bf16
  hx     = [Q_j*h | Q_j | pad] per head             [j, 8*66]  exit+scale
  eT     = max(ub*v_j, 1) * msk    per head         [j, NCH*N] DVE only
  agg    : num/den via eT-stationary matmuls        PE
  hh     = num * (1/den); z = ELU(hh)
  layer 2 identical with g = z @ Wo (+tl/tr cols), u2/v2/Q2 from ACT exps
  out    = ELU(a2 @ g / Z2) + x
"""

import numpy as np
import ml_dtypes
from contextlib import ExitStack

BF16 = ml_dtypes.bfloat16
B, N, F, D, H, C = 8, 1024, 256, 64, 8, 256
HD = H * D  # 512
ALPHA = 0.2

_CACHE = {}


def _build_program():
    import concourse.bacc as bacc
    import concourse.bass as bass
    import concourse.mybir as mybir
    from concourse.tile import TileContext
    from concourse.masks import make_identity

    dt = mybir.dt
    Alu = mybir.AluOpType
    Act = mybir.ActivationFunctionType

    nc = bacc.Bacc()

    xt = nc.declare_dram_parameter("xt", [F + 1, N], dt.bfloat16, isOutput=False)
    xs = nc.declare_dram_parameter("xs", [N, F], dt.float32, isOutput=False)
    msk = nc.declare_dram_parameter("msk", [N, N], dt.bfloat16, isOutput=False)
    wp = nc.declare_dram_parameter("wp", [F + 1, HD], dt.bfloat16, isOutput=False)
    ut = nc.declare_dram_parameter("ut", [H, N], dt.bfloat16, isOutput=False)
    vq = nc.declare_dram_parameter("vq", [N, 2 * H], dt.float32, isOutput=False)
    wo = nc.declare_dram_parameter("wo", [HD + 1, C + 2], dt.bfloat16, isOutput=False)
    out_d = nc.declare_dram_parameter("out", [N, C], dt.float32, isOutput=True)

    rows_d = nc.dram_tensor("rows_bounce", [1, N], dt.bfloat16)

    NCH = N // 128  # 8 chunks of 128 nodes

    def bcast128(row_ap):
        # [1, N] DRAM row -> [128, N] partition-broadcast read for DMA
        return bass.AP(
            tensor=row_ap.tensor,
            offset=row_ap.offset,
            ap=[[0, 128]] + list(row_ap.ap),
        )

    with TileContext(nc) as tc:
        with ExitStack() as ctx:
            cons = ctx.enter_context(tc.tile_pool(name="cons", bufs=1))
            bc = ctx.enter_context(tc.tile_pool(name="bc", bufs=3))
            eb = ctx.enter_context(tc.tile_pool(name="eb", bufs=5))
            wk = ctx.enter_context(tc.tile_pool(name="wk", bufs=3))
            sm = ctx.enter_context(tc.tile_pool(name="sm", bufs=3))
            pmm = ctx.enter_context(tc.tile_pool(name="pmm", bufs=4, space="PSUM"))
            pm2 = ctx.enter_context(tc.tile_pool(name="pm2", bufs=2, space="PSUM"))
            ptp = ctx.enter_context(tc.tile_pool(name="ptp", bufs=1, space="PSUM"))

            # ---------- constants / params ----------
            ident_f = cons.tile([128, 128], dt.float32)
            make_identity(nc, ident_f[:, :])
            ident_b = cons.tile([128, 128], dt.bfloat16)
            make_identity(nc, ident_b[:, :])

            # mask first (the first head's tensor_tensor waits on it), split
            # across two engine queues for parallel descriptor generation
            msk_sb = cons.tile([128, NCH * N], dt.bfloat16)
            for c in range(NCH):
                eng = nc.gpsimd if c % 2 == 0 else nc.scalar
                eng.dma_start(
                    out=msk_sb[:, c * N : (c + 1) * N],
                    in_=msk[c * 128 : (c + 1) * 128, :],
                )

            xt_sb = cons.tile([128, 2 * N], dt.bfloat16)
            nc.gpsimd.dma_start(out=xt_sb[:, 0:N], in_=xt[0:128, :])
            nc.gpsimd.dma_start(out=xt_sb[:, N : 2 * N], in_=xt[128:256, :])
            xt_one = cons.tile([1, N], dt.bfloat16)
            nc.gpsimd.dma_start(out=xt_one[:, :], in_=xt[256:257, :])

            wp_sb = cons.tile([128, 2 * HD], dt.bfloat16)
            nc.gpsimd.dma_start(out=wp_sb[:, 0:HD], in_=wp[0:128, :])
            nc.gpsimd.dma_start(out=wp_sb[:, HD : 2 * HD], in_=wp[128:256, :])
            wp_one = cons.tile([1, HD], dt.bfloat16)
            nc.gpsimd.dma_start(out=wp_one[:, :], in_=wp[256:257, :])

            wo_sb = cons.tile([128, 4 * (C + 2)], dt.bfloat16)
            for k in range(4):
                nc.gpsimd.dma_start(
                    out=wo_sb[:, k * (C + 2) : (k + 1) * (C + 2)],
                    in_=wo[k * 128 : (k + 1) * 128, :],
                )
            wo_one = cons.tile([1, C + 2], dt.bfloat16)
            nc.gpsimd.dma_start(out=wo_one[:, :], in_=wo[HD : HD + 1, :])

            # residual input, prefetched early so the output tail doesn't
            # wait on per-chunk DMA round trips
            xs_sb = cons.tile([128, NCH * F], dt.float32)
            for c in range(NCH):
                nc.scalar.dma_start(
                    out=xs_sb[:, c * F : (c + 1) * F],
                    in_=xs[c * 128 : (c + 1) * 128, :],
                )

            # v / Q per-partition scalars: vqt[p, n, 0:8]=v_h, [.., 8:16]=Q_h
            vqt = cons.tile([128, NCH * 2 * H], dt.float32)
            nc.sync.dma_start(
                out=vqt[:, :].rearrange("p (n h) -> p n h", h=2 * H),
                in_=vq[:, :].rearrange("(n p) h -> p n h", p=128),
            )

            # ---------- phase 1: h = x@W_all, scaled by Q ----------
            hx = cons.tile([128, NCH * 8 * 66], dt.bfloat16)  # [Qh(64)|Q|pad]/head
            nc.vector.memset(
                hx[:, :].rearrange("p (n h s) -> p n h s", h=8, s=66)[:, :, :, 64:66],
                1.0,
            )
            qrep = cons.tile([128, NCH * 528], dt.bfloat16)
            for n in range(NCH):
                nc.scalar.activation(
                    qrep[:, n * 528 : (n + 1) * 528].rearrange(
                        "p (h s) -> p h s", s=66
                    ),
                    vqt[:, :]
                    .rearrange("p (n h) -> p n h", h=2 * H)[:, n : n + 1, H : 2 * H]
                    .rearrange("p n h -> p h n")
                    .to_broadcast([128, 8, 66]),
                    Act.Copy,
                )

            def emit_h_proj(n0, n1):
                for n in range(n0, n1):
                    ph = pmm.tile([128, HD], dt.float32, tag="mm")
                    for k in range(2):
                        lt = xt_sb[:, k * N + n * 128 : k * N + n * 128 + 128]
                        nc.tensor.matmul(
                            ph[:, :], lt, wp_sb[:, k * HD : (k + 1) * HD],
                            start=(k == 0), stop=False,
                        )
                    lt1 = xt_one[:, n * 128 : n * 128 + 128]
                    nc.tensor.matmul(
                        ph[:, :], lt1, wp_one[:, :], start=False, stop=True
                    )
                    # exit h -> hx (bf16, 66-stride blocks; Q cols pre-set 1)
                    hx_v = hx[:, n * 528 : (n + 1) * 528].rearrange(
                        "p (h s) -> p h s", s=66
                    )[:, :, 0:64]
                    ph_v = ph[:, :].rearrange("p (h s) -> p h s", s=64)
                    nc.scalar.activation(hx_v, ph_v, Act.Copy)
                    # scale whole 528-block by Q (ones col -> Q, pad -> junk)
                    nc.vector.tensor_tensor(
                        out=hx[:, n * 528 : (n + 1) * 528],
                        in0=hx[:, n * 528 : (n + 1) * 528],
                        in1=qrep[:, n * 528 : (n + 1) * 528],
                        op=Alu.mult,
                    )

            # ---------- phase 2: eT = max(ub*v,1)*msk per head; agg ----------
            zacc = cons.tile([128, NCH * 512], dt.bfloat16)
            rz_all = cons.tile([128, NCH * 8], dt.float32)
            z_sb = cons.tile([128, NCH * HD], dt.bfloat16)

            def build_e(ub, v_col):
                """eT[j, i] = max(u_i * v_j, 1) * msk[j, i]."""
                e = eb.tile([128, NCH * N], dt.bfloat16, tag="e")
                for c in range(NCH):
                    nc.vector.tensor_scalar(
                        out=e[:, c * N : (c + 1) * N], in0=ub[:, :],
                        scalar1=v_col(c), scalar2=1.0,
                        op0=Alu.mult, op1=Alu.max,
                    )
                HN = NCH * N // 2
                for half in range(2):
                    nc.vector.tensor_tensor(
                        out=e[:, half * HN : (half + 1) * HN].bitcast(dt.int16),
                        in0=e[:, half * HN : (half + 1) * HN].bitcast(dt.int16),
                        in1=msk_sb[:, half * HN : (half + 1) * HN].bitcast(dt.int16),
                        op=Alu.bitwise_and,
                    )
                return e

            for q in range(2):
                ebufs = []
                for hq in range(4):
                    h = 4 * q + hq
                    ub = bc.tile([128, N], dt.bfloat16, tag="ub")
                    nc.sync.dma_start(out=ub[:, :], in_=bcast128(ut[h : h + 1, :]))
                    e = build_e(
                        ub,
                        lambda c, h=h: vqt[:, c * 2 * H + h : c * 2 * H + h + 1],
                    )
                    ebufs.append(e)
                    if q == 0:
                        emit_h_proj(2 * hq, 2 * hq + 2)

                for ic in range(NCH):
                    pa = pmm.tile([128, 260], dt.float32, tag="mm")
                    for hq in range(4):
                        h = 4 * q + hq
                        e = ebufs[hq]
                        for jc in range(NCH):
                            nc.tensor.matmul(
                                pa[:, hq * 65 : hq * 65 + 65],
                                e[:, jc * N + ic * 128 : jc * N + ic * 128 + 128],
                                hx[:, jc * 528 + h * 66 : jc * 528 + h * 66 + 65],
                                start=(jc == 0),
                                stop=(jc == NCH - 1),
                            )
                    nc.scalar.activation(
                        zacc[:, ic * 512 + q * 256 : ic * 512 + q * 256 + 256]
                        .rearrange("p (h s) -> p h s", s=64),
                        pa[:, :].rearrange("p (h s) -> p h s", s=65)[:, :, 0:64],
                        Act.Copy,
                    )
                    nc.vector.reciprocal(
                        rz_all[:, ic * 8 + q * 4 : ic * 8 + q * 4 + 4]
                        .rearrange("p (h s) -> p h s", s=1),
                        pa[:, :].rearrange("p (h s) -> p h s", s=65)[:, :, 64:65],
                    )

            # ---------- phase 2b-4: per-chunk divide/ELU -> zT -> g ----------
            zt_sb = cons.tile([128, 4 * N], dt.bfloat16)
            zt_one = cons.tile([1, N], dt.bfloat16)
            nc.vector.memset(zt_one[:, :], 1.0)
            gx = cons.tile([128, NCH * 260], dt.bfloat16)
            nc.vector.memset(
                gx[:, :].rearrange("p (n s) -> p n s", s=260)[:, :, 256:257], 1.0
            )
            glgr = cons.tile([128, NCH * 2], dt.float32)
            v2q2 = cons.tile([128, NCH * 2], dt.float32)
            pt2 = ptp.tile([2, N], dt.float32, tag="tp")
            for ic in range(NCH):
                rzrep = wk.tile([128, HD], dt.bfloat16, tag="rzrep")
                hh = wk.tile([128, HD], dt.bfloat16, tag="hh")
                nc.scalar.activation(
                    rzrep[:, :].rearrange("p (h s) -> p h s", s=64),
                    rz_all[:, ic * 8 : (ic + 1) * 8]
                    .rearrange("p (h s) -> p h s", s=1)
                    .to_broadcast([128, 8, 64]),
                    Act.Copy,
                )
                nc.vector.tensor_tensor(
                    out=hh[:, :],
                    in0=zacc[:, ic * 512 : (ic + 1) * 512],
                    in1=rzrep[:, :],
                    op=Alu.mult,
                )
                ee = wk.tile([128, HD], dt.bfloat16, tag="ee")
                nc.scalar.activation(ee[:, :], hh[:, :], Act.Exp)
                r1 = wk.tile([128, HD], dt.bfloat16, tag="r1")
                nc.vector.tensor_scalar(
                    out=r1[:, :], in0=ee[:, :], scalar1=-1.0, scalar2=0.0,
                    op0=Alu.add, op1=Alu.min,
                )
                nc.vector.scalar_tensor_tensor(
                    out=z_sb[:, ic * HD : (ic + 1) * HD],
                    in0=hh[:, :], scalar=0.0, in1=r1[:, :],
                    op0=Alu.max, op1=Alu.add,
                )

                # zT for this node chunk (4 transposed 128x128 blocks)
                pzi = pm2.tile([128, 4 * 128], dt.bfloat16, tag="mm2")
                for kc in range(4):
                    nc.tensor.transpose(
                        pzi[:, kc * 128 : (kc + 1) * 128],
                        z_sb[:, ic * HD + kc * 128 : ic * HD + kc * 128 + 128],
                        ident_b[:, :],
                    )
                nc.scalar.activation(
                    zt_sb[:, :]
                    .rearrange("p (kc n) -> p kc n", n=N)[:, :, ic * 128 : ic * 128 + 128],
                    pzi[:, :].rearrange("p (kc s) -> p kc s", s=128),
                    Act.Copy,
                )

                # g projection for this chunk: g = z@Wo (+tl/tr cols)
                pg = pm2.tile([128, C + 2], dt.float32, tag="mm2")
                for kc in range(4):
                    nc.tensor.matmul(
                        pg[:, :],
                        zt_sb[:, kc * N + ic * 128 : kc * N + ic * 128 + 128],
                        wo_sb[:, kc * (C + 2) : (kc + 1) * (C + 2)],
                        start=(kc == 0), stop=False,
                    )
                nc.tensor.matmul(
                    pg[:, :], zt_one[:, ic * 128 : ic * 128 + 128], wo_one[:, :],
                    start=False, stop=True,
                )
                nc.scalar.activation(
                    gx[:, ic * 260 : ic * 260 + C], pg[:, 0:C], Act.Copy
                )
                nc.vector.tensor_copy(
                    out=glgr[:, ic * 2 : (ic + 1) * 2], in_=pg[:, C : C + 2]
                )
                # v2 = exp(-0.8*tr), Q2 = exp(tr) per-partition scalars
                nc.scalar.activation(
                    v2q2[:, ic * 2 : ic * 2 + 1],
                    glgr[:, ic * 2 + 1 : ic * 2 + 2],
                    Act.Exp, scale=-0.8,
                )
                nc.scalar.activation(
                    v2q2[:, ic * 2 + 1 : ic * 2 + 2],
                    glgr[:, ic * 2 + 1 : ic * 2 + 2],
                    Act.Exp,
                )
                # scale g block by Q2 (ones col -> Q2)
                nc.vector.tensor_scalar(
                    out=gx[:, ic * 260 : ic * 260 + C + 1],
                    in0=gx[:, ic * 260 : ic * 260 + C + 1],
                    scalar1=v2q2[:, ic * 2 + 1 : ic * 2 + 2], scalar2=None,
                    op0=Alu.mult,
                )
                # tl row form for this chunk
                nc.tensor.transpose(
                    pt2[:, ic * 128 : (ic + 1) * 128],
                    glgr[:, ic * 2 : (ic + 1) * 2],
                    ident_f[:, :],
                )

            # u2 row = exp(-0.8 * tl) -> DRAM bounce -> bcast
            u2row = cons.tile([1, N], dt.bfloat16)
            nc.scalar.activation(u2row[:, :], pt2[0:1, :], Act.Exp, scale=-0.8)
            nc.sync.dma_start(out=rows_d[0:1, :], in_=u2row[:, :])

            # ---------- phase 5: output attention layer ----------
            ub2 = bc.tile([128, N], dt.bfloat16, tag="ub")
            nc.sync.dma_start(out=ub2[:, :], in_=bcast128(rows_d[0:1, :]))
            e2 = build_e(ub2, lambda c: v2q2[:, c * 2 : c * 2 + 1])

            for ic in range(NCH):
                po = pmm.tile([128, C + 1], dt.float32, tag="mm")
                for jc in range(NCH):
                    nc.tensor.matmul(
                        po[:, :],
                        e2[:, jc * N + ic * 128 : jc * N + ic * 128 + 128],
                        gx[:, jc * 260 : jc * 260 + C + 1],
                        start=(jc == 0), stop=(jc == NCH - 1),
                    )
                rz2 = sm.tile([128, 1], dt.float32, tag="rz2")
                nc.vector.reciprocal(rz2[:, :], po[:, C : C + 1])
                y = sm.tile([128, C], dt.bfloat16, tag="y")
                nc.vector.tensor_scalar(
                    out=y[:, :], in0=po[:, 0:C], scalar1=rz2[:, :], scalar2=None,
                    op0=Alu.mult,
                )
                e3 = sm.tile([128, C], dt.bfloat16, tag="e3")
                nc.scalar.activation(e3[:, :], y[:, :], Act.Exp)
                r2 = sm.tile([128, C], dt.bfloat16, tag="r2")
                nc.vector.tensor_scalar(
                    out=r2[:, :], in0=e3[:, :], scalar1=-1.0, scalar2=0.0,
                    op0=Alu.add, op1=Alu.min,
                )
                el = sm.tile([128, C], dt.bfloat16, tag="el")
                nc.vector.scalar_tensor_tensor(
                    out=el[:, :], in0=y[:, :], scalar=0.0, in1=r2[:, :],
                    op0=Alu.max, op1=Alu.add,
                )
                ofin = sm.tile([128, C], dt.float32, tag="ofin")
                nc.vector.tensor_tensor(
                    out=ofin[:, :], in0=el[:, :],
                    in1=xs_sb[:, ic * F : (ic + 1) * F], op=Alu.add,
                )
                nc.sync.dma_start(
                    out=out_d[ic * 128 : (ic + 1) * 128, :], in_=ofin[:, :]
                )

    nc.compile()
    return nc


def get_program():
    if "nc" not in _CACHE:
        _CACHE["nc"] = _build_program()
    return _CACHE["nc"]


def make_in_maps(x, adj, W, Wb, a, ab, Wo, Wob, ao, aob):
    x = np.asarray(x, np.float32)
    adj = np.asarray(adj)
    W = np.asarray(W, np.float32)
    Wb = np.asarray(Wb, np.float32)
    a = np.asarray(a, np.float32)
    ab = np.asarray(ab, np.float32)
    Wo = np.asarray(Wo, np.float32)
    Wob = np.asarray(Wob, np.float32)
    ao = np.asarray(ao, np.float32)
    aob = np.asarray(aob, np.float32)

    # W_all[f, h*D+d] = W[h, f, d];  Wb row flattened the same way
    W_all = W.transpose(1, 0, 2).reshape(F, HD)
    wb_row = Wb.reshape(1, HD)
    wp = np.concatenate([W_all, wb_row], axis=0).astype(BF16)  # [257, 512]

    # sl/sr are tiny per-node linear maps of x — folded on the host.
    # sl[b, h, i] = x[b,i] @ V_l[:,h] + const_l[h]
    # sr[b, h, j] likewise; ab folded into sl
    V_l = np.einsum("hfd,hd->fh", W, a[:, :D]).astype(np.float32)
    V_r = np.einsum("hfd,hd->fh", W, a[:, D:]).astype(np.float32)
    const_l = (Wb * a[:, :D]).sum(1) + ab  # [H]
    const_r = (Wb * a[:, D:]).sum(1)
    sl_all = np.einsum("bnf,fh->bhn", x, V_l) + const_l[None, :, None]  # [B,H,N]
    sr_all = np.einsum("bnf,fh->bnh", x, V_r) + const_r[None, None, :]  # [B,N,H]

    # separable softmax factors (e^{sl} cancels in num/den)
    u_all = np.exp(-0.8 * sl_all)            # [B,H,N]
    v_all = np.exp(-0.8 * sr_all)            # [B,N,H]
    q_all = np.exp(sr_all)                   # [B,N,H]
    vq_all = np.concatenate([v_all, q_all], axis=2).astype(np.float32)  # [B,N,2H]

    u_l = Wo @ ao[:C]  # [512]
    u_r = Wo @ ao[C:]
    wo_top = np.concatenate([Wo, u_l[:, None], u_r[:, None]], axis=1)  # [512, 258]
    wo_bot = np.concatenate(
        [Wob, [Wob @ ao[:C] + aob], [Wob @ ao[C:]]]
    )[None, :]  # [1, 258]
    wo_ext = np.concatenate([wo_top, wo_bot], axis=0).astype(BF16)  # [513, 258]

    ones_row = np.ones((1, N), BF16)
    in_maps = []
    for b in range(B):
        xt = np.concatenate([x[b].T.astype(BF16), ones_row], axis=0)  # [257, 1024]
        mb = np.where(adj[b].T > 0, np.uint16(0xFFFF), np.uint16(0)).view(BF16)
        in_maps.append(
            {
                "xt": np.ascontiguousarray(xt),
                "xs": np.ascontiguousarray(x[b]),
                "msk": np.ascontiguousarray(mb),
                "wp": wp,
                "ut": np.ascontiguousarray(u_all[b].astype(BF16)),
                "vq": np.ascontiguousarray(vq_all[b]),
                "wo": wo_ext,
            }
        )
    return in_maps


def kernel(**inputs) -> np.ndarray:
    from concourse.bass_utils import run_bass_kernel_spmd

    nc = get_program()
    in_maps = make_in_maps(**inputs)
    res = run_bass_kernel_spmd(nc, in_maps, core_ids=list(range(B)))
    return np.stack([res.results[b]["out"] for b in range(B)], axis=0)
